# revision 2
# baseline (speedup 1.0000x reference)
"""CompGCN layer (TransE composition, mean aggregation, 3-way linear + BatchNorm)
as a Trainium2 Bass/Tile kernel on 8 NeuronCores — v2.

Sharding: nodes range-sharded across 8 cores (12544 slots each, snake-deal
balanced by degree).  Each core processes the edges whose aggregation key
(dst for the forward pass, src for the reverse pass) falls in its node range.

v2 design vs baseline:
- One indirect gather per node tile ([128, nch] offset AP -> nch*128
  descriptors in a single SWDGE instruction) instead of one per 128-edge
  chunk: ~7x fewer Pool-engine descriptor-generation holds.
- Aggregation matmuls run "flipped": lhsT = gathered features / edge
  embeddings (bf16), rhs = degree-weighted one-hot (bf16), accumulating
  feature-major [f, node] tiles in PSUM.  No PE transposes anywhere; BN
  statistics and affine become per-partition scalar ops; the output is
  stored feature-major and un-transposed on the host.
- 1/deg edge weights are folded into the one-hot (host-computed from the
  int index data), so mean aggregation needs no count column.
- Node/edge payloads travel as bf16; edge streams are packed two chunks
  per 512B descriptor.
- Per-tile chunk counts are variable (static max across cores) instead of
  a uniform cmax; each 7-tile group is padded to an even chunk count so
  the paired edge-stream DMA stays 512B-aligned.
- The three projections consume the feature-major accumulators directly
  (weights are staged as W.T), and BatchNorm stats (sum, sum of squares)
  accumulate per-tile via fused reduce ops, all-reduced as a [128, 2]
  tensor across cores.

Bias adds and the /3 are algebraically dropped: BatchNorm's mean
subtraction cancels any per-feature constant shift, and its variance
normalization cancels any global scale.
"""
import sys
sys.path.insert(0, "/opt/trn_rl_repo")

import numpy as np

import concourse.bass as bass
import concourse.mybir as mybir
import concourse.tile as tile
from concourse.bass import IndirectOffsetOnAxis
from concourse.bass_utils import run_bass_kernel_spmd
from concourse.masks import make_identity

P = 128
D = 128
N_CORES = 8
N_NODES = 100000
N_EDGES = 600000
NPC = 12544            # padded nodes per core (98 tiles of 128)
NT = NPC // P          # node tiles per core
NPAD = N_CORES * NPC   # padded global node count
GT = 7                 # tiles per DMA group
NG = NT // GT          # groups
BN_EPS = 1e-5
F32 = mybir.dt.float32
BF16 = mybir.dt.bfloat16
I32 = mybir.dt.int32
PAD_KLOC = 200.0       # one-hot never matches -> padded edges contribute nothing
N_SWDGE_Q = 4
MAX_NCH = 8            # ring capacity: nch*128 descriptors must stay <= 1024


def _split_multi_waits(nc):
    """This walrus build encodes at most one sync wait per instruction; hoist
    extra waits onto single-wait NoOps just before the instruction (same
    engine, same queue order - semantics unchanged)."""
    for func in nc.m.functions:
        for bb in func.blocks:
            new_instrs = []
            for ins in bb.instructions:
                si = ins.sync_info
                waits = list(si.on_wait) if (si is not None and si.on_wait) else []
                if len(waits) > 1:
                    for k, w in enumerate(waits[:-1]):
                        new_instrs.append(mybir.InstNoOp(
                            name=f"{ins.name}.sw{k}", engine=ins.engine,
                            ins=[], outs=[],
                            sync_info=mybir.SyncInfo(on_wait=[w], on_update=[]),
                        ))
                    ins.sync_info = mybir.SyncInfo(
                        on_wait=[waits[-1]], on_update=list(si.on_update or []))
                new_instrs.append(ins)
            bb.instructions = new_instrs


def _spread_swdge_queues(nc):
    """Round-robin the indirect gathers over the SWDGE queues (the builder
    emits them all on qPoolDynamic; parallel queues overlap desc-gen/transfer)."""
    k = 0
    for func in nc.m.functions:
        for bb in func.blocks:
            for ins in bb.instructions:
                if (type(ins).__name__ == "InstDMACopy"
                        and getattr(ins, "queue", None) == "qPoolDynamic"):
                    q = k % N_SWDGE_Q
                    k += 1
                    if q:
                        ins.queue = f"qPoolDynamic{q}"


def _chunk_layout(nch):
    """Static chunk bookkeeping for one pass: per-tile chunk counts (already
    group-evened), prefix starts, group chunk ranges, and the max group width."""
    nch = list(nch)
    assert len(nch) == NT
    cstart = np.concatenate(([0], np.cumsum(nch))).astype(int)
    C = int(cstart[-1])
    gb = [int(cstart[g * GT]) for g in range(NG)] + [C]
    for g in range(NG):
        assert (gb[g + 1] - gb[g]) % 2 == 0, "group chunk counts must be even"
    wmax = max(gb[g + 1] - gb[g] for g in range(NG))
    return nch, cstart, C, gb, wmax


def build_program(nch_o, nch_i, rep=1, ablate=(), debug_taps=False):
    """ablate: subset of {"onehot","mm","gather","estr"} — skip those
    instruction classes (timing-attribution sims only; breaks numerics).
    debug_taps: add diagnostic ExternalOutputs (ho_accT, stats, one xg)."""
    ablate = set(ablate)
    nc = bass.Bass("TRN2", num_devices=N_CORES, debug=False,
                   num_swdge_queues=N_SWDGE_Q)

    nch_o, cs_o, C_o, gb_o, wm_o = _chunk_layout(nch_o)
    nch_i, cs_i, C_i, gb_i, wm_i = _chunk_layout(nch_i)
    wmax = max(wm_o, wm_i)
    nchmax = max(max(nch_o), max(nch_i))
    assert nchmax <= MAX_NCH

    xpadb = nc.dram_tensor("xpadb", [NPAD, D], BF16, kind="ExternalInput")
    ixo = nc.dram_tensor("ixo", [P, 3 * C_o], I32, kind="ExternalInput")
    ixi = nc.dram_tensor("ixi", [P, 3 * C_i], I32, kind="ExternalInput")
    eo2 = nc.dram_tensor("eo2", [(C_o // 2) * P, 2 * D], BF16,
                         kind="ExternalInput")
    ei2 = nc.dram_tensor("ei2", [(C_i // 2) * P, 2 * D], BF16,
                         kind="ExternalInput")
    xot = nc.dram_tensor("xot", [D, NPC], F32, kind="ExternalInput")
    wot = nc.dram_tensor("wot", [D, D], F32, kind="ExternalInput")
    wit = nc.dram_tensor("wit", [D, D], F32, kind="ExternalInput")
    wst = nc.dram_tensor("wst", [D, D], F32, kind="ExternalInput")
    gbp = nc.dram_tensor("gbp", [D, 2], F32, kind="ExternalInput")
    outT = nc.dram_tensor("outT", [D, NPC], F32, kind="ExternalOutput")
    if debug_taps:
        dbg_ho = nc.dram_tensor("dbg_ho", [P, NT * P], F32,
                                kind="ExternalOutput")
        dbg_h = nc.dram_tensor("dbg_h", [P, NT * P], F32,
                               kind="ExternalOutput")
        dbg_st = nc.dram_tensor("dbg_st", [P, 2 * NT + 2], F32,
                                kind="ExternalOutput")
        dbg_xg = nc.dram_tensor("dbg_xg", [P, MAX_NCH * P], BF16,
                                kind="ExternalOutput")

    with tile.TileContext(nc) as tc:
        with tc.tile_pool(name="persist", bufs=1) as pp, \
             tc.tile_pool(name="dram", bufs=1, space="DRAM") as dp:
            iota_f = pp.tile([P, P], F32, tag="iota_f")
            iota_i = pp.tile([P, P], I32, tag="iota_i")
            nc.gpsimd.iota(iota_i[:], pattern=[[1, P]], base=0,
                           channel_multiplier=0)
            nc.vector.tensor_copy(iota_f[:], iota_i[:])
            w_t = {}
            for nm, dt_ in (("wot", wot), ("wit", wit), ("wst", wst)):
                w_t[nm] = pp.tile([D, D], F32, tag=nm, name=f"w_{nm}")
                nc.sync.dma_start(w_t[nm][:], dt_.ap())
            gb_sb = pp.tile([P, 2], F32, tag="gb_sb")
            nc.sync.dma_start(gb_sb[:], gbp.ap())
            epsb = pp.tile([P, 1], F32, tag="epsb")
            nc.vector.memset(epsb[:], BN_EPS)

            ho_accT = pp.tile([P, NT * P], F32, tag="ho_accT")
            h_accT = pp.tile([P, NT * P], F32, tag="h_accT")
            s1col = pp.tile([P, NT], F32, tag="s1col")
            s2col = pp.tile([P, NT], F32, tag="s2col")

            cin = dp.tile([P, 2], F32)
            cout = dp.tile([P, 2], F32)

            for _ in range(rep):
                for pas, (ixd, ed2, nch, cstart, C, gbounds) in enumerate((
                        (ixo, eo2, nch_o, cs_o, C_o, gb_o),
                        (ixi, ei2, nch_i, cs_i, C_i, gb_i))):
                    with tc.tile_pool(name="agg_ix", bufs=1) as ixp, \
                         tc.tile_pool(name="agg_io", bufs=3) as io, \
                         tc.tile_pool(name="agg_oh", bufs=6) as ohp, \
                         tc.tile_pool(name="agg_ps", bufs=2, space="PSUM") as ps, \
                         tc.tile_pool(name="agg_pj", bufs=2, space="PSUM") as pj:
                        ixsb = ixp.tile([P, 3 * C], I32, tag="ixsb")
                        nc.sync.dma_start(ixsb[:], ixd.ap())
                        ixf = ixsb[:].bitcast(F32)
                        for g in range(NG):
                            c0, c1 = gbounds[g], gbounds[g + 1]
                            W = c1 - c0
                            estr = io.tile([P, wmax * P], BF16, tag="estr")
                            if "estr" not in ablate:
                                nc.sync.dma_start(
                                    estr[:, :W * P].rearrange(
                                        "p (h f) -> p h f", f=2 * D),
                                    ed2.ap()[(c0 // 2) * P:(c1 // 2) * P, :]
                                        .rearrange("(h p) f -> p h f", p=P))
                            if pas == 1:
                                xog = io.tile([P, GT * P], F32, tag="xog")
                                nc.sync.dma_start(
                                    xog[:],
                                    xot.ap()[:, g * GT * P:(g + 1) * GT * P])
                            # per-chunk gathers ([128,1] offsets): the only
                            # vector-indirect form this walrus build lowers
                            # correctly (multi-column offset APs silently
                            # become "fetch idx and idx+1" row pairs)
                            xgs = {}
                            for cc in range(c0, c1):
                                xc = io.tile([P, P], BF16, tag="xc", bufs=10)
                                if "gather" not in ablate:
                                    nc.gpsimd.indirect_dma_start(
                                        out=xc[:], out_offset=None,
                                        in_=xpadb.ap()[:, :],
                                        in_offset=IndirectOffsetOnAxis(
                                            ap=ixsb[:, cc:cc + 1], axis=0))
                                xgs[cc] = xc
                            if debug_taps and pas == 0 and g == 0:
                                for h in range(8):
                                    nc.sync.dma_start(
                                        dbg_xg.ap()[:, h * P:(h + 1) * P],
                                        xgs[c0 + h][:])
                            for u in range(GT):
                                t = g * GT + u
                                tc0 = int(cstart[t])
                                n = nch[t]
                                agg = ps.tile([P, P], F32, tag="agg")
                                for j in range(n):
                                    c = tc0 + j
                                    jl = c - c0
                                    oh = ohp.tile([P, P], BF16, tag="oh")
                                    if "onehot" not in ablate:
                                        nc.vector.tensor_scalar(
                                            out=oh[:], in0=iota_f[:],
                                            scalar1=ixf[:, C + c:C + c + 1],
                                            scalar2=ixf[:, 2 * C + c:2 * C + c + 1],
                                            op0=mybir.AluOpType.is_equal,
                                            op1=mybir.AluOpType.mult)
                                    if "mm" not in ablate:
                                        # edge stream is host-negated, so both
                                        # matmuls accumulate x + (-e)
                                        nc.tensor.matmul(
                                            agg[:],
                                            lhsT=xgs[c][:],
                                            rhs=oh[:],
                                            start=(j == 0), stop=False)
                                        nc.tensor.matmul(
                                            agg[:],
                                            lhsT=estr[:, jl * P:(jl + 1) * P],
                                            rhs=oh[:],
                                            start=False, stop=(j == n - 1))
                                if pas == 0:
                                    nc.vector.tensor_copy(
                                        ho_accT[:, t * P:(t + 1) * P], agg[:])
                                else:
                                    hi = io.tile([P, P], F32, tag="hi")
                                    nc.vector.tensor_copy(hi[:], agg[:])
                                    hp = pj.tile([P, P], F32, tag="hp")
                                    nc.tensor.matmul(
                                        hp[:], lhsT=w_t["wot"][:],
                                        rhs=ho_accT[:, t * P:(t + 1) * P],
                                        start=True, stop=False)
                                    nc.tensor.matmul(
                                        hp[:], lhsT=w_t["wit"][:], rhs=hi[:],
                                        start=False, stop=False)
                                    nc.tensor.matmul(
                                        hp[:], lhsT=w_t["wst"][:],
                                        rhs=xog[:, u * P:(u + 1) * P],
                                        start=False, stop=True)
                                    hs = h_accT[:, t * P:(t + 1) * P]
                                    nc.vector.tensor_copy(hs, hp[:])
                                    nc.vector.tensor_reduce(
                                        s1col[:, t:t + 1], hs,
                                        axis=mybir.AxisListType.X,
                                        op=mybir.AluOpType.add)
                                    sqd = io.tile([P, P], F32, tag="sqd")
                                    nc.scalar.square(sqd[:], hp[:])
                                    nc.vector.tensor_reduce(
                                        s2col[:, t:t + 1], sqd[:],
                                        axis=mybir.AxisListType.X,
                                        op=mybir.AluOpType.add)

                # ---- global BN stats + affine ----
                with tc.tile_pool(name="bn_io", bufs=2) as io:
                    stats = io.tile([P, 2], F32, tag="stats")
                    nc.vector.tensor_reduce(
                        stats[:, 0:1], s1col[:], axis=mybir.AxisListType.X,
                        op=mybir.AluOpType.add)
                    nc.vector.tensor_reduce(
                        stats[:, 1:2], s2col[:], axis=mybir.AxisListType.X,
                        op=mybir.AluOpType.add)
                    nc.gpsimd.dma_start(cin[:], stats[:])
                    nc.gpsimd.collective_compute(
                        "AllReduce", mybir.AluOpType.add,
                        replica_groups=[list(range(N_CORES))],
                        ins=[cin.opt()], outs=[cout.opt()])
                    gs = io.tile([P, 2], F32, tag="gs")
                    nc.sync.dma_start(gs[:], cout[:])
                    if debug_taps:
                        nc.sync.dma_start(dbg_ho.ap(), ho_accT[:])
                        nc.sync.dma_start(dbg_h.ap(), h_accT[:])
                        nc.sync.dma_start(dbg_st.ap()[:, 0:NT], s1col[:])
                        nc.sync.dma_start(dbg_st.ap()[:, NT:2 * NT], s2col[:])
                        nc.sync.dma_start(dbg_st.ap()[:, 2 * NT:2 * NT + 2],
                                          gs[:])
                    mu = io.tile([P, 1], F32, tag="mu")
                    nc.vector.tensor_scalar_mul(mu[:], gs[:, 0:1], 1.0 / N_NODES)
                    ex2 = io.tile([P, 1], F32, tag="ex2")
                    nc.vector.tensor_scalar_mul(ex2[:], gs[:, 1:2], 1.0 / N_NODES)
                    mu2 = io.tile([P, 1], F32, tag="mu2")
                    nc.vector.tensor_mul(mu2[:], mu[:], mu[:])
                    var = io.tile([P, 1], F32, tag="var")
                    nc.vector.tensor_sub(var[:], ex2[:], mu2[:])
                    sd = io.tile([P, 1], F32, tag="sd")
                    nc.scalar.activation(sd[:], var[:],
                                         mybir.ActivationFunctionType.Sqrt,
                                         bias=epsb[:])
                    inv = io.tile([P, 1], F32, tag="inv")
                    nc.vector.reciprocal(inv[:], sd[:])
                    A = io.tile([P, 1], F32, tag="A")
                    nc.vector.tensor_mul(A[:], inv[:], gb_sb[:, 0:1])
                    muA = io.tile([P, 1], F32, tag="muA")
                    nc.vector.tensor_mul(muA[:], mu[:], A[:])
                    B = io.tile([P, 1], F32, tag="B")
                    nc.vector.tensor_sub(B[:], gb_sb[:, 1:2], muA[:])

                    with tc.tile_pool(name="st_io", bufs=2) as so:
                        for g in range(NG):
                            ob = so.tile([P, GT * P], F32, tag="ob")
                            for u in range(GT):
                                t = g * GT + u
                                nc.vector.tensor_scalar(
                                    out=ob[:, u * P:(u + 1) * P],
                                    in0=h_accT[:, t * P:(t + 1) * P],
                                    scalar1=A[:, 0:1], scalar2=B[:, 0:1],
                                    op0=mybir.AluOpType.mult,
                                    op1=mybir.AluOpType.add)
                            nc.sync.dma_start(
                                outT.ap()[:, g * GT * P:(g + 1) * GT * P],
                                ob[:])

    return nc


def _balance_perm(src, dst, core):
    """Snake-deal the core's nodes into tiles by total degree so per-tile edge
    loads are near-uniform.  Returns pos[node_local] -> slot."""
    base = core * NPC
    deg = np.zeros(NPC, np.int64)
    for key in (src, dst):
        sel = key[(key >= base) & (key < base + NPC)] - base
        deg += np.bincount(sel, minlength=NPC)
    ranks = np.argsort(-deg, kind="stable")
    r = np.arange(NPC)
    sweep, lane = r // NT, r % NT
    tile_of_rank = np.where(sweep % 2 == 0, lane, NT - 1 - lane)
    pos = np.empty(NPC, np.int64)
    pos[ranks] = tile_of_rank * P + sweep
    return pos


def _prep_pass(key, gat, core, pos, rdeg_of_key):
    """Index-only host prep for one (core, pass): map the aggregation key to
    its balanced slot, sort the core's edge shard by slot (then by gather
    index within each tile for locality), and return per-tile runs."""
    base = core * NPC
    sel = np.nonzero((key >= base) & (key < base + NPC))[0]
    k = pos[key[sel] - base]
    order = np.lexsort((gat[sel], k >> 7))
    k = k[order]
    g = gat[sel][order]
    e = sel[order]
    w = rdeg_of_key[key[sel]][order].astype(np.float32)
    tile_id = (k >> 7).astype(np.int64)
    cnt = np.bincount(tile_id, minlength=NT)
    return k, g, e, w, tile_id, cnt


def prepare_in_maps(inputs):
    return _prepare_in_maps(**inputs)


def _prepare_in_maps(node_embs, edge_embs, W_O, b_O, W_I, b_I, W_S, b_S,
                     gamma, beta, src, dst):
    import ml_dtypes
    node_embs = np.asarray(node_embs, np.float32)
    edge_embs_b = np.asarray(edge_embs, np.float32).astype(ml_dtypes.bfloat16)
    src = np.asarray(src).astype(np.int64)
    dst = np.asarray(dst).astype(np.int64)

    xpad = np.zeros((NPAD, D), np.float32)
    xpad[:N_NODES] = node_embs
    xpadb = xpad.astype(ml_dtypes.bfloat16)

    deg_o = np.bincount(dst, minlength=NPAD).astype(np.float64)
    deg_i = np.bincount(src, minlength=NPAD).astype(np.float64)
    rdeg_o = (1.0 / np.maximum(deg_o, 1.0)).astype(np.float32)
    rdeg_i = (1.0 / np.maximum(deg_i, 1.0)).astype(np.float32)

    passes = {}
    poss = []
    cnts = {"o": np.zeros((N_CORES, NT), np.int64),
            "i": np.zeros((N_CORES, NT), np.int64)}
    for c in range(N_CORES):
        pos = _balance_perm(src, dst, c)
        poss.append(pos)
        for nm, key, gat, rd in (("o", dst, src, rdeg_o),
                                 ("i", src, dst, rdeg_i)):
            pp = _prep_pass(key, gat, c, pos, rd)
            passes[(c, nm)] = pp
            cnts[nm][c] = pp[5]

    # static per-tile chunk counts: max across cores, then even per group
    nchs = {}
    for nm in ("o", "i"):
        nch = np.maximum((cnts[nm].max(axis=0) + P - 1) // P, 1).astype(int)
        for g in range(NG):
            if nch[g * GT:(g + 1) * GT].sum() % 2:
                nch[(g + 1) * GT - 1] += 1
        assert nch.max() <= MAX_NCH, f"nch overflow: {nch.max()}"
        nchs[nm] = nch
    print(f"kernel2: C_o={int(nchs['o'].sum())} C_i={int(nchs['i'].sum())} "
          f"chunks/pass (vs uniform {NT * 7})")

    in_maps = []
    for c in range(N_CORES):
        inv_pos = np.argsort(poss[c])
        xo = xpad[c * NPC:(c + 1) * NPC][inv_pos]
        m = {
            "xpadb": xpadb,
            "xot": np.ascontiguousarray(xo.T),
            "wot": np.ascontiguousarray(W_O.T).astype(np.float32),
            "wit": np.ascontiguousarray(W_I.T).astype(np.float32),
            "wst": np.ascontiguousarray(W_S.T).astype(np.float32),
            "gbp": np.ascontiguousarray(
                np.stack([np.asarray(gamma, np.float32),
                          np.asarray(beta, np.float32)], axis=1)),
        }
        for nm in ("o", "i"):
            k, g, e, w, tile_id, cnt = passes[(c, nm)]
            nch = nchs[nm]
            cstart = np.concatenate(([0], np.cumsum(nch))).astype(np.int64)
            C = int(cstart[-1])
            # slot within the padded chunk space for each edge
            run_start = np.concatenate(([0], np.cumsum(cnt)[:-1]))
            off = np.arange(len(k)) - run_start[tile_id]
            slot = cstart[tile_id] * P + off
            gidx = np.zeros((C * P,), np.int32)
            klocf = np.full((C * P,), PAD_KLOC, np.float32)
            redge = np.zeros((C * P,), np.float32)
            eid = np.full((C * P,), -1, np.int64)
            # chunk-major slots: chunk c slot p at c*P + p; edge `off` within
            # tile t -> chunk cstart[t] + off//P, lane off%P
            dest = (cstart[tile_id] + off // P) * P + (off % P)
            gidx[dest] = g
            klocf[dest] = (k & 127).astype(np.float32)
            redge[dest] = w
            eid[dest] = e
            # ix tensor [128, 3C]: lane-major views (partition = lane)
            ix = np.empty((P, 3 * C), np.int32)
            ix[:, 0:C] = gidx.reshape(C, P).T
            ix[:, C:2 * C] = klocf.view(np.int32).reshape(C, P).T
            ix[:, 2 * C:3 * C] = redge.view(np.int32).reshape(C, P).T
            m["ix" + nm] = np.ascontiguousarray(ix)
            # paired edge stream [(C//2)*P, 256] bf16, negated so the device
            # accumulates x + (-e) in a single PSUM group
            est = np.zeros((C * P, D), ml_dtypes.bfloat16)
            real = eid >= 0
            est[real] = -edge_embs_b[eid[real]]
            m["e" + nm + "2"] = np.ascontiguousarray(
                est.reshape(C // 2, 2, P, D).transpose(0, 2, 1, 3)
                   .reshape((C // 2) * P, 2 * D))
        in_maps.append(m)
    return in_maps, nchs, poss


def assemble_output(per_core_outT, poss):
    """Transpose back to node-major, undo the balance permutation, trim pads."""
    h = np.concatenate(
        [np.asarray(per_core_outT[c]).T[poss[c]] for c in range(N_CORES)],
        axis=0)
    return h[:N_NODES].astype(np.float32)


def kernel(**inputs):
    in_maps, nchs, poss = prepare_in_maps(inputs)
    nc = build_program(list(nchs["o"]), list(nchs["i"]))
    _spread_swdge_queues(nc)
    _split_multi_waits(nc)
    res = run_bass_kernel_spmd(nc, in_maps, core_ids=list(range(N_CORES)),
                               trace=False)
    return assemble_output([res.results[c]["outT"] for c in range(N_CORES)],
                           poss)
